# revision 43
# baseline (speedup 1.0000x reference)
# Binary linear: y[b,s,o] = sum_i x[b,s,i] * sign(W)[o,i]
#
# Strategy (8 NeuronCores, data-parallel over tokens):
#   - Host: flatten x to [32768, 768], shard 8 x [4096, 768], pre-transpose each
#     shard to xT [768, 4096] (contraction dim i must sit on SBUF partitions for
#     the PE matmul; doing the transpose on host avoids burning PE/DVE cycles).
#   - Host: pre-transpose W to wT [i, o] (replicated to all cores).
#   - Device (per core): sign(wT) on ACT -> wbinT [i, o]; stream xT strips;
#     for each 128-token block accumulate over 6 i-blocks:
#       psum[t,o] += xT_tile[i,t].T @ wbinT[i,o]   (f32r matmuls: full PE rate,
#     exact +-1 weights so only x suffers the reduced-precision rounding)
#   - PSUM -> SBUF copies split across DVE/ACT, DMA y [4096, 768] back.
#   - Host: concat shards -> [4, 8192, 768].

import numpy as np

N_CORES = 8
B, S, D_IN, D_OUT = 4, 8192, 768, 768
T_TOTAL = B * S            # 32768 tokens
T_CORE = T_TOTAL // N_CORES  # 4096 tokens per core
P = 128
IB = D_IN // P             # 6 i-blocks (contraction)
TB = T_CORE // P           # 32 token-blocks per core
# graduated chunk widths: small first chunks so the PE can start early,
# large later chunks for DMA efficiency. The late chunks are issued
# mid-loop so their streams don't starve the y-store DMA completions.
CHUNKS = [256, 512, 1280, 1024, 1024]
assert sum(CHUNKS) == T_CORE
O_SPLIT = 512              # split for the PSUM->SBUF copies (DVE/ACT balance)

_cache = {}


def _build():
    import concourse.bacc as bacc
    import concourse.mybir as mybir
    import concourse.tile as tile

    f32 = mybir.dt.float32
    bf16 = mybir.dt.bfloat16

    nc = bacc.Bacc(
        "TRN2",
        target_bir_lowering=False,
        debug=False,
        num_devices=N_CORES,
    )

    xT = nc.dram_tensor("xT", [D_IN, T_CORE], bf16, kind="ExternalInput")
    wT = nc.dram_tensor("wT", [D_IN, D_OUT], bf16, kind="ExternalInput")
    y = nc.dram_tensor("y", [T_CORE, D_OUT], f32, kind="ExternalOutput")

    with tile.TileContext(nc) as tc:
        with (
            tc.tile_pool(name="wbin", bufs=1) as wbin_pool,
            tc.tile_pool(name="xbuf", bufs=1) as x_pool,
            tc.tile_pool(name="ybuf", bufs=8) as y_pool,
            tc.tile_pool(name="psum", bufs=3, space="PSUM") as psum_pool,
        ):
            chunk_start = []
            s = 0
            for w_ in CHUNKS:
                chunk_start.append(s)
                s += w_

            # --- PE warmup: dummy matmuls on zeroed scratch during the
            # framework preamble / first DMAs, so the HAM clock gate is at
            # full rate (K=8/8) when the real matmuls start. ---
            wu = x_pool.tile([P, 640], bf16, tag="warmup", name="wu")
            nc.vector.memset(wu[:], 0.0)
            wups = psum_pool.tile([P, O_SPLIT], f32, tag="wups", name="wups", bufs=1)
            for k in range(12):
                nc.tensor.matmul(
                    wups[:], wu[:, :P], wu[:, P:P + O_SPLIT],
                    start=True, stop=True, skip_group_check=True,
                )
            wu_out = x_pool.tile([P, O_SPLIT], f32, tag="warmup_out", name="wu_out")
            nc.vector.tensor_copy(wu_out[:], wups[:])

            # Pre-binarized (+-1) weight strips loaded across all three DMA
            # rings (ACT + SP HWDGE, gpsimd SWDGE), two strips per DMA so
            # each ring pays a single completion receipt.
            wbin = [None] * IB

            def w_load(pair, eng):
                wb = wbin_pool.tile(
                    [P, 2 * D_OUT], bf16, tag=f"wbin{pair}", name=f"wbin{pair}"
                )
                eng.dma_start(
                    wb.rearrange("p (b o) -> p b o", b=2),
                    wT[2 * pair * P : (2 * pair + 2) * P, :].rearrange(
                        "(b p) o -> p b o", p=P
                    ),
                )
                wbin[2 * pair] = wb[:, :D_OUT]
                wbin[2 * pair + 1] = wb[:, D_OUT:]

            xch = [None] * len(CHUNKS)

            def x_load(c):
                cw = CHUNKS[c]
                c0 = chunk_start[c]
                xt = x_pool.tile([P, IB * cw], bf16, tag=f"xc{c}", name=f"xc{c}")
                nc.sync.dma_start(
                    xt.rearrange("p (b t) -> p b t", b=IB),
                    xT[:, c0 : c0 + cw].rearrange("(b p) t -> p b t", p=P),
                )
                xch[c] = xt

            x_load(0)
            w_load(0, nc.scalar)
            w_load(1, nc.gpsimd)
            w_load(2, nc.sync)
            x_load(1)
            x_load(2)

            def chunk_of(tok):
                for c in range(len(CHUNKS) - 1, -1, -1):
                    if tok >= chunk_start[c]:
                        return c, tok - chunk_start[c]
                raise AssertionError

            # --- main loop: one 128-token block at a time ---
            for j in range(TB):
                if j == 4:
                    x_load(3)
                elif j == 10:
                    x_load(4)
                c, off = chunk_of(j * P)
                cw = CHUNKS[c]
                ps = psum_pool.tile([P, D_OUT], f32, tag="ps", name=f"ps{j}")
                for i in range(IB):
                    lhsT = xch[c][:, i * cw + off : i * cw + off + P]
                    # one matmul may not cross a PSUM bank (512 f32); split 512+256
                    nc.tensor.matmul(
                        ps[:, :O_SPLIT],
                        lhsT,
                        wbin[i][:, :O_SPLIT],
                        start=(i == 0),
                        stop=(i == IB - 1),
                    )
                    nc.tensor.matmul(
                        ps[:, O_SPLIT:],
                        lhsT,
                        wbin[i][:, O_SPLIT:],
                        start=(i == 0),
                        stop=(i == IB - 1),
                    )
                yt = y_pool.tile([P, D_OUT], f32, tag="y", name=f"y{j}")
                nc.vector.tensor_copy(yt[:, :O_SPLIT], ps[:, :O_SPLIT])
                nc.scalar.copy(yt[:, O_SPLIT:], ps[:, O_SPLIT:])
                nc.sync.dma_start(y[j * P : (j + 1) * P, :], yt[:])

    nc.compile()
    return nc


def _get_nc():
    if "nc" not in _cache:
        _cache["nc"] = _build()
    return _cache["nc"]


def _prep_inputs(x, weight):
    import ml_dtypes

    x = np.asarray(x, dtype=np.float32)
    w = np.asarray(weight, dtype=np.float32)
    x2 = x.reshape(N_CORES, T_CORE, D_IN)
    # bf16 shards, transposed so the contraction dim is on partitions
    xT_shards = np.ascontiguousarray(
        x2.transpose(0, 2, 1).astype(ml_dtypes.bfloat16)
    )  # [8, 768, 4096] bf16
    # replicate the small binarized weight (per the data-parallel sharding):
    # +-1 (and 0) are exact in bf16
    wT = np.ascontiguousarray(np.sign(w).T.astype(ml_dtypes.bfloat16))  # [i, o]
    return [
        {"xT": xT_shards[c], "wT": wT} for c in range(N_CORES)
    ]


def _install_axon_ntff_hook():
    """The agent image's `antenv` lacks `axon_hooks`; register an equivalent
    module backed by direct ctypes calls into libaxon_pjrt.so so that
    run_bass_kernel_spmd(trace=True) can capture NTFF profiles under axon."""
    import sys

    if "antenv.axon_hooks" in sys.modules:
        return
    import contextlib
    import ctypes
    import types

    so_path = "/opt/axon/libaxon_pjrt.so"
    try:
        lib = ctypes.CDLL(so_path)
    except OSError:
        return
    if not hasattr(lib, "axon_start_nrt_profile"):
        return
    lib.axon_start_nrt_profile.argtypes = [
        ctypes.POINTER(ctypes.c_int64),
        ctypes.c_size_t,
    ]
    lib.axon_start_nrt_profile.restype = ctypes.c_int64
    lib.axon_stop_nrt_profile.argtypes = [ctypes.c_char_p]
    lib.axon_stop_nrt_profile.restype = ctypes.c_int64

    @contextlib.contextmanager
    def _hook(output_dir, device_ids):
        import jax

        jax.devices()
        if device_ids:
            ids = (ctypes.c_int64 * len(device_ids))(*device_ids)
            rc = lib.axon_start_nrt_profile(ids, len(device_ids))
        else:
            rc = lib.axon_start_nrt_profile(None, 0)
        if rc != 0:
            raise RuntimeError(f"axon_start_nrt_profile rc={rc}")
        try:
            yield
        finally:
            n = lib.axon_stop_nrt_profile(str(output_dir).encode())
            print(f"ntff profile: {n} file(s) written to {output_dir}")

    mod = types.ModuleType("antenv.axon_hooks")
    mod.get_axon_ntff_profile_hook = lambda: _hook
    mod.set_axon_ntff_profile_hook = lambda h: None
    sys.modules["antenv.axon_hooks"] = mod


def _run(x, weight, trace=False):
    from concourse.bass_utils import run_bass_kernel_spmd

    if trace:
        _install_axon_ntff_hook()
    nc = _get_nc()
    in_maps = _prep_inputs(x, weight)
    res = run_bass_kernel_spmd(
        nc, in_maps, core_ids=list(range(N_CORES)), trace=trace
    )
    y_full = np.concatenate([r["y"] for r in res.results], axis=0)
    return y_full.reshape(B, S, D_OUT), res


def kernel(x, weight):
    out, _ = _run(x, weight, trace=False)
    return out


# revision 46
# speedup vs baseline: 1.0026x; 1.0026x over previous
# Binary linear: y[b,s,o] = sum_i x[b,s,i] * sign(W)[o,i]
#
# Strategy (8 NeuronCores, data-parallel over tokens):
#   - Host: flatten x to [32768, 768], shard 8 x [4096, 768], pre-transpose
#     each shard to xT [768, 4096] bf16 (the contraction dim must sit on SBUF
#     partitions for the PE matmul; host-side transpose avoids burning
#     PE/DVE cycles, bf16 halves the HBM read traffic; weights are exactly
#     +-1 so only x carries the bf16 rounding, rel err ~1.7e-3).
#   - Host: replicate the small binarized weight (per the data-parallel
#     sharding) as wT = sign(W).T bf16 [i, o].
#   - Device (per core): PE-warmup matmuls during the framework preamble
#     (HAM clock gate); weight strips spread over all 3 DMA rings; x loaded
#     in graduated chunks (small first so matmuls start early, big later
#     chunks issued mid-loop so they don't starve y-store completions);
#     per 128-token block accumulate over 6 i-blocks:
#       psum[t,o] += xT_tile[i,t].T @ wbinT[i,o]   (bf16, N=512+256 per bank)
#     then PSUM -> SBUF copies split across DVE/ACT and y DMA out (f32).
#   - Host: concat shards -> [4, 8192, 768] f32.
#
# Measured on 8x trn2 NeuronCores: ~83.3 us HW exec (slowest core),
# rel err 1.7e-3. PE matmul-stream floor is 61.4 us; DMA ~20 MB/core.

import numpy as np

N_CORES = 8
B, S, D_IN, D_OUT = 4, 8192, 768, 768
T_TOTAL = B * S            # 32768 tokens
T_CORE = T_TOTAL // N_CORES  # 4096 tokens per core
P = 128
IB = D_IN // P             # 6 i-blocks (contraction)
TB = T_CORE // P           # 32 token-blocks per core
# graduated chunk widths: small first chunks so the PE can start early,
# large later chunks for DMA efficiency. The late chunks are issued
# mid-loop so their streams don't starve the y-store DMA completions.
CHUNKS = [256, 512, 1280, 1024, 1024]
assert sum(CHUNKS) == T_CORE
O_SPLIT = 512              # split for the PSUM->SBUF copies (DVE/ACT balance)

_cache = {}


def _build():
    import concourse.bacc as bacc
    import concourse.mybir as mybir
    import concourse.tile as tile

    f32 = mybir.dt.float32
    bf16 = mybir.dt.bfloat16

    nc = bacc.Bacc(
        "TRN2",
        target_bir_lowering=False,
        debug=False,
        num_devices=N_CORES,
    )

    xT = nc.dram_tensor("xT", [D_IN, T_CORE], bf16, kind="ExternalInput")
    wT = nc.dram_tensor("wT", [D_IN, D_OUT], bf16, kind="ExternalInput")
    y = nc.dram_tensor("y", [T_CORE, D_OUT], f32, kind="ExternalOutput")

    with tile.TileContext(nc) as tc:
        with (
            tc.tile_pool(name="wbin", bufs=1) as wbin_pool,
            tc.tile_pool(name="xbuf", bufs=1) as x_pool,
            tc.tile_pool(name="ybuf", bufs=8) as y_pool,
            tc.tile_pool(name="psum", bufs=3, space="PSUM") as psum_pool,
        ):
            chunk_start = []
            s = 0
            for w_ in CHUNKS:
                chunk_start.append(s)
                s += w_

            # --- PE warmup: dummy matmuls on zeroed scratch during the
            # framework preamble / first DMAs, so the HAM clock gate is at
            # full rate (K=8/8) when the real matmuls start. ---
            wu = x_pool.tile([P, 640], bf16, tag="warmup", name="wu")
            nc.vector.memset(wu[:], 0.0)
            wups = psum_pool.tile([P, O_SPLIT], f32, tag="wups", name="wups", bufs=1)
            for k in range(12):
                nc.tensor.matmul(
                    wups[:], wu[:, :P], wu[:, P:P + O_SPLIT],
                    start=True, stop=True, skip_group_check=True,
                )
            wu_out = x_pool.tile([P, O_SPLIT], f32, tag="warmup_out", name="wu_out")
            nc.vector.tensor_copy(wu_out[:], wups[:])

            # Pre-binarized (+-1) weight strips loaded across all three DMA
            # rings (ACT + SP HWDGE, gpsimd SWDGE) so every strip lands early.
            wbin = [None] * IB

            def w_load(i, eng):
                wb = wbin_pool.tile([P, D_OUT], bf16, tag=f"wbin{i}", name=f"wbin{i}")
                eng.dma_start(wb[:], wT[i * P : (i + 1) * P, :])
                wbin[i] = wb

            xch = [None] * len(CHUNKS)

            def x_load(c):
                cw = CHUNKS[c]
                c0 = chunk_start[c]
                xt = x_pool.tile([P, IB * cw], bf16, tag=f"xc{c}", name=f"xc{c}")
                nc.sync.dma_start(
                    xt.rearrange("p (b t) -> p b t", b=IB),
                    xT[:, c0 : c0 + cw].rearrange("(b p) t -> p b t", p=P),
                )
                xch[c] = xt

            x_load(0)
            w_load(0, nc.scalar)
            w_load(1, nc.gpsimd)
            w_load(2, nc.sync)
            w_load(3, nc.scalar)
            w_load(4, nc.gpsimd)
            w_load(5, nc.scalar)
            x_load(1)
            x_load(2)

            def chunk_of(tok):
                for c in range(len(CHUNKS) - 1, -1, -1):
                    if tok >= chunk_start[c]:
                        return c, tok - chunk_start[c]
                raise AssertionError

            # --- main loop: one 128-token block at a time ---
            for j in range(TB):
                if j == 4:
                    x_load(3)
                elif j == 10:
                    x_load(4)
                c, off = chunk_of(j * P)
                cw = CHUNKS[c]
                ps = psum_pool.tile([P, D_OUT], f32, tag="ps", name=f"ps{j}")
                for i in range(IB):
                    lhsT = xch[c][:, i * cw + off : i * cw + off + P]
                    # one matmul may not cross a PSUM bank (512 f32); split 512+256
                    nc.tensor.matmul(
                        ps[:, :O_SPLIT],
                        lhsT,
                        wbin[i][:, :O_SPLIT],
                        start=(i == 0),
                        stop=(i == IB - 1),
                    )
                    nc.tensor.matmul(
                        ps[:, O_SPLIT:],
                        lhsT,
                        wbin[i][:, O_SPLIT:],
                        start=(i == 0),
                        stop=(i == IB - 1),
                    )
                yt = y_pool.tile([P, D_OUT], f32, tag="y", name=f"y{j}")
                nc.vector.tensor_copy(yt[:, :O_SPLIT], ps[:, :O_SPLIT])
                nc.scalar.copy(yt[:, O_SPLIT:], ps[:, O_SPLIT:])
                nc.sync.dma_start(y[j * P : (j + 1) * P, :], yt[:])

    nc.compile()
    return nc


def _get_nc():
    if "nc" not in _cache:
        _cache["nc"] = _build()
    return _cache["nc"]


def _prep_inputs(x, weight):
    import ml_dtypes

    x = np.asarray(x, dtype=np.float32)
    w = np.asarray(weight, dtype=np.float32)
    x2 = x.reshape(N_CORES, T_CORE, D_IN)
    # bf16 shards, transposed so the contraction dim is on partitions
    xT_shards = np.ascontiguousarray(
        x2.transpose(0, 2, 1).astype(ml_dtypes.bfloat16)
    )  # [8, 768, 4096] bf16
    # replicate the small binarized weight (per the data-parallel sharding):
    # +-1 (and 0) are exact in bf16
    wT = np.ascontiguousarray(np.sign(w).T.astype(ml_dtypes.bfloat16))  # [i, o]
    return [
        {"xT": xT_shards[c], "wT": wT} for c in range(N_CORES)
    ]


def _install_axon_ntff_hook():
    """The agent image's `antenv` lacks `axon_hooks`; register an equivalent
    module backed by direct ctypes calls into libaxon_pjrt.so so that
    run_bass_kernel_spmd(trace=True) can capture NTFF profiles under axon."""
    import sys

    if "antenv.axon_hooks" in sys.modules:
        return
    import contextlib
    import ctypes
    import types

    so_path = "/opt/axon/libaxon_pjrt.so"
    try:
        lib = ctypes.CDLL(so_path)
    except OSError:
        return
    if not hasattr(lib, "axon_start_nrt_profile"):
        return
    lib.axon_start_nrt_profile.argtypes = [
        ctypes.POINTER(ctypes.c_int64),
        ctypes.c_size_t,
    ]
    lib.axon_start_nrt_profile.restype = ctypes.c_int64
    lib.axon_stop_nrt_profile.argtypes = [ctypes.c_char_p]
    lib.axon_stop_nrt_profile.restype = ctypes.c_int64

    @contextlib.contextmanager
    def _hook(output_dir, device_ids):
        import jax

        jax.devices()
        if device_ids:
            ids = (ctypes.c_int64 * len(device_ids))(*device_ids)
            rc = lib.axon_start_nrt_profile(ids, len(device_ids))
        else:
            rc = lib.axon_start_nrt_profile(None, 0)
        if rc != 0:
            raise RuntimeError(f"axon_start_nrt_profile rc={rc}")
        try:
            yield
        finally:
            n = lib.axon_stop_nrt_profile(str(output_dir).encode())
            print(f"ntff profile: {n} file(s) written to {output_dir}")

    mod = types.ModuleType("antenv.axon_hooks")
    mod.get_axon_ntff_profile_hook = lambda: _hook
    mod.set_axon_ntff_profile_hook = lambda h: None
    sys.modules["antenv.axon_hooks"] = mod


def _run(x, weight, trace=False):
    from concourse.bass_utils import run_bass_kernel_spmd

    if trace:
        _install_axon_ntff_hook()
    nc = _get_nc()
    in_maps = _prep_inputs(x, weight)
    res = run_bass_kernel_spmd(
        nc, in_maps, core_ids=list(range(N_CORES)), trace=trace
    )
    y_full = np.concatenate([r["y"] for r in res.results], axis=0)
    return y_full.reshape(B, S, D_OUT), res


def kernel(x, weight):
    out, _ = _run(x, weight, trace=False)
    return out


# revision 47
# speedup vs baseline: 1.0092x; 1.0066x over previous
# Binary linear: y[b,s,o] = sum_i x[b,s,i] * sign(W)[o,i]
#
# Strategy (8 NeuronCores, data-parallel over tokens):
#   - Host: flatten x to [32768, 768], shard 8 x [4096, 768], pre-transpose
#     each shard to xT [768, 4096] bf16 (the contraction dim must sit on SBUF
#     partitions for the PE matmul; host-side transpose avoids burning
#     PE/DVE cycles, bf16 halves the HBM read traffic; weights are exactly
#     +-1 so only x carries the bf16 rounding, rel err ~1.7e-3).
#   - Host: replicate the small binarized weight (per the data-parallel
#     sharding) as wT = sign(W).T bf16 [i, o].
#   - Device (per core): PE-warmup matmuls during the framework preamble
#     (HAM clock gate); weight strips spread over all 3 DMA rings; x loaded
#     in graduated chunks (small first so matmuls start early, big later
#     chunks issued mid-loop so they don't starve y-store completions);
#     per 128-token block accumulate over 6 i-blocks:
#       psum[t,o] += xT_tile[i,t].T @ wbinT[i,o]   (bf16, N=512+256 per bank)
#     then PSUM -> SBUF copies split across DVE/ACT and y DMA out (f32).
#   - Host: concat shards -> [4, 8192, 768] f32.
#
# Measured on 8x trn2 NeuronCores: ~83.3 us HW exec (slowest core),
# rel err 1.7e-3. PE matmul-stream floor is 61.4 us; DMA ~20 MB/core.

import numpy as np

N_CORES = 8
B, S, D_IN, D_OUT = 4, 8192, 768, 768
T_TOTAL = B * S            # 32768 tokens
T_CORE = T_TOTAL // N_CORES  # 4096 tokens per core
P = 128
IB = D_IN // P             # 6 i-blocks (contraction)
TB = T_CORE // P           # 32 token-blocks per core
# graduated chunk widths: small first chunks so the PE can start early,
# large later chunks for DMA efficiency. The late chunks are issued
# mid-loop so their streams don't starve the y-store DMA completions.
CHUNKS = [256, 512, 1280, 1024, 1024]
assert sum(CHUNKS) == T_CORE
O_SPLIT = 512              # split for the PSUM->SBUF copies (DVE/ACT balance)

_cache = {}


def _build():
    import concourse.bacc as bacc
    import concourse.mybir as mybir
    import concourse.tile as tile

    f32 = mybir.dt.float32
    bf16 = mybir.dt.bfloat16

    nc = bacc.Bacc(
        "TRN2",
        target_bir_lowering=False,
        debug=False,
        num_devices=N_CORES,
    )

    xT = nc.dram_tensor("xT", [D_IN, T_CORE], bf16, kind="ExternalInput")
    wT = nc.dram_tensor("wT", [D_IN, D_OUT], bf16, kind="ExternalInput")
    y = nc.dram_tensor("y", [T_CORE, D_OUT], f32, kind="ExternalOutput")

    with tile.TileContext(nc) as tc:
        with (
            tc.tile_pool(name="wbin", bufs=1) as wbin_pool,
            tc.tile_pool(name="xbuf", bufs=1) as x_pool,
            tc.tile_pool(name="ybuf", bufs=8) as y_pool,
            tc.tile_pool(name="psum", bufs=3, space="PSUM") as psum_pool,
        ):
            chunk_start = []
            s = 0
            for w_ in CHUNKS:
                chunk_start.append(s)
                s += w_

            # --- PE warmup: dummy matmuls on zeroed scratch during the
            # framework preamble / first DMAs, so the HAM clock gate is at
            # full rate (K=8/8) when the real matmuls start. ---
            wu = x_pool.tile([P, 640], bf16, tag="warmup", name="wu")
            nc.vector.memset(wu[:], 0.0)
            wups = psum_pool.tile([P, O_SPLIT], f32, tag="wups", name="wups", bufs=1)
            for k in range(12):
                nc.tensor.matmul(
                    wups[:], wu[:, :P], wu[:, P:P + O_SPLIT],
                    start=True, stop=True, skip_group_check=True,
                )
            wu_out = x_pool.tile([P, O_SPLIT], f32, tag="warmup_out", name="wu_out")
            nc.vector.tensor_copy(wu_out[:], wups[:])

            # Pre-binarized (+-1) weight strips loaded across all three DMA
            # rings (ACT + SP HWDGE, gpsimd SWDGE) so every strip lands early.
            wbin = [None] * IB

            def w_load(i, eng):
                wb = wbin_pool.tile([P, D_OUT], bf16, tag=f"wbin{i}", name=f"wbin{i}")
                eng.dma_start(wb[:], wT[i * P : (i + 1) * P, :])
                wbin[i] = wb

            xch = [None] * len(CHUNKS)

            def x_load(c):
                cw = CHUNKS[c]
                c0 = chunk_start[c]
                xt = x_pool.tile([P, IB * cw], bf16, tag=f"xc{c}", name=f"xc{c}")
                nc.sync.dma_start(
                    xt.rearrange("p (b t) -> p b t", b=IB),
                    xT[:, c0 : c0 + cw].rearrange("(b p) t -> p b t", p=P),
                )
                xch[c] = xt

            x_load(0)
            w_load(0, nc.scalar)
            w_load(1, nc.gpsimd)
            w_load(2, nc.sync)
            w_load(3, nc.scalar)
            w_load(4, nc.gpsimd)
            w_load(5, nc.scalar)
            x_load(1)
            x_load(2)

            def chunk_of(tok):
                for c in range(len(CHUNKS) - 1, -1, -1):
                    if tok >= chunk_start[c]:
                        return c, tok - chunk_start[c]
                raise AssertionError

            # --- main loop: one 128-token block at a time ---
            for j in range(TB):
                if j == 4:
                    x_load(3)
                elif j == 10:
                    x_load(4)
                c, off = chunk_of(j * P)
                cw = CHUNKS[c]
                ps = psum_pool.tile([P, D_OUT], f32, tag="ps", name=f"ps{j}")
                for i in range(IB):
                    lhsT = xch[c][:, i * cw + off : i * cw + off + P]
                    # one matmul may not cross a PSUM bank (512 f32); split 512+256
                    nc.tensor.matmul(
                        ps[:, :O_SPLIT],
                        lhsT,
                        wbin[i][:, :O_SPLIT],
                        start=(i == 0),
                        stop=(i == IB - 1),
                    )
                    nc.tensor.matmul(
                        ps[:, O_SPLIT:],
                        lhsT,
                        wbin[i][:, O_SPLIT:],
                        start=(i == 0),
                        stop=(i == IB - 1),
                    )
                yt = y_pool.tile([P, D_OUT], f32, tag="y", name=f"y{j}")
                nc.vector.tensor_copy(yt[:, :O_SPLIT], ps[:, :O_SPLIT])
                nc.scalar.copy(yt[:, O_SPLIT:], ps[:, O_SPLIT:])
                if j >= TB - 2:
                    # tail: store each half as soon as its copy lands, on
                    # separate rings, so the final receipts overlap
                    nc.sync.dma_start(
                        y[j * P : (j + 1) * P, :O_SPLIT], yt[:, :O_SPLIT]
                    )
                    nc.scalar.dma_start(
                        y[j * P : (j + 1) * P, O_SPLIT:], yt[:, O_SPLIT:]
                    )
                else:
                    eng = nc.sync if j % 2 == 0 else nc.scalar
                    eng.dma_start(y[j * P : (j + 1) * P, :], yt[:])

    nc.compile()
    return nc


def _get_nc():
    if "nc" not in _cache:
        _cache["nc"] = _build()
    return _cache["nc"]


def _prep_inputs(x, weight):
    import ml_dtypes

    x = np.asarray(x, dtype=np.float32)
    w = np.asarray(weight, dtype=np.float32)
    x2 = x.reshape(N_CORES, T_CORE, D_IN)
    # bf16 shards, transposed so the contraction dim is on partitions
    xT_shards = np.ascontiguousarray(
        x2.transpose(0, 2, 1).astype(ml_dtypes.bfloat16)
    )  # [8, 768, 4096] bf16
    # replicate the small binarized weight (per the data-parallel sharding):
    # +-1 (and 0) are exact in bf16
    wT = np.ascontiguousarray(np.sign(w).T.astype(ml_dtypes.bfloat16))  # [i, o]
    return [
        {"xT": xT_shards[c], "wT": wT} for c in range(N_CORES)
    ]


def _install_axon_ntff_hook():
    """The agent image's `antenv` lacks `axon_hooks`; register an equivalent
    module backed by direct ctypes calls into libaxon_pjrt.so so that
    run_bass_kernel_spmd(trace=True) can capture NTFF profiles under axon."""
    import sys

    if "antenv.axon_hooks" in sys.modules:
        return
    import contextlib
    import ctypes
    import types

    so_path = "/opt/axon/libaxon_pjrt.so"
    try:
        lib = ctypes.CDLL(so_path)
    except OSError:
        return
    if not hasattr(lib, "axon_start_nrt_profile"):
        return
    lib.axon_start_nrt_profile.argtypes = [
        ctypes.POINTER(ctypes.c_int64),
        ctypes.c_size_t,
    ]
    lib.axon_start_nrt_profile.restype = ctypes.c_int64
    lib.axon_stop_nrt_profile.argtypes = [ctypes.c_char_p]
    lib.axon_stop_nrt_profile.restype = ctypes.c_int64

    @contextlib.contextmanager
    def _hook(output_dir, device_ids):
        import jax

        jax.devices()
        if device_ids:
            ids = (ctypes.c_int64 * len(device_ids))(*device_ids)
            rc = lib.axon_start_nrt_profile(ids, len(device_ids))
        else:
            rc = lib.axon_start_nrt_profile(None, 0)
        if rc != 0:
            raise RuntimeError(f"axon_start_nrt_profile rc={rc}")
        try:
            yield
        finally:
            n = lib.axon_stop_nrt_profile(str(output_dir).encode())
            print(f"ntff profile: {n} file(s) written to {output_dir}")

    mod = types.ModuleType("antenv.axon_hooks")
    mod.get_axon_ntff_profile_hook = lambda: _hook
    mod.set_axon_ntff_profile_hook = lambda h: None
    sys.modules["antenv.axon_hooks"] = mod


def _run(x, weight, trace=False):
    from concourse.bass_utils import run_bass_kernel_spmd

    if trace:
        _install_axon_ntff_hook()
    nc = _get_nc()
    in_maps = _prep_inputs(x, weight)
    res = run_bass_kernel_spmd(
        nc, in_maps, core_ids=list(range(N_CORES)), trace=trace
    )
    y_full = np.concatenate([r["y"] for r in res.results], axis=0)
    return y_full.reshape(B, S, D_OUT), res


def kernel(x, weight):
    out, _ = _run(x, weight, trace=False)
    return out
